# revision 55
# baseline (speedup 1.0000x reference)
"""Llama GQA attention (B=2, S=2048, D=2048, NH=16, NKV=4, HD=128) on 8 TRN2 cores.

Sharding: core c = batch*4 + kv_group  (2 batches x 4 kv groups).
Each core computes 4 q heads + 1 kv head for one batch, then a partial
output projection with its 512-row slice of Wo; the host sums the 4
partials per batch.

Device-side layout trick: everything is computed in "transposed"
orientation (QT/KT = [head_dim, S]) so that
  - projections take x^T tiles as the moving operand (host supplies x^T),
  - scoresT[sk, sq] = KT_tile^T @ QT needs no transposes,
  - softmax exp runs on scoresT, denominators come from ones^T @ expT
    matmuls (col-packed via tile_position), and
  - attn_outT[d, sq] accumulates with lhsT = V tile in natural [s, d]
    layout, rhs = expT; attn_outT then feeds the Wo matmul as lhsT.
Causal structure: score/AV/denominator matmuls are only emitted for
lower-triangular (sk, sq) tile pairs; the 128x128 diagonal blocks are
masked with a precomputed 0/1 triangle after exp.

Per-chunk schedule: all score matmuls + exps complete first (expT tiles
buffered in SBUF), then the AV + denominator matmuls stream back-to-back
on the PE with no ACT dependency, then the normalize (reciprocal ->
partition-broadcast -> multiply) runs while the next chunk's scores start.

RoPE: host permutes Wq/Wk columns within each head to evens-then-odds, so
the interleaved-pair rotation becomes contiguous half-rotations on the
partition dim; q.k dot products are invariant to the (shared) permutation.
Softmax max-subtraction is skipped: scores/sqrt(128) are O(10), exp is
safe in f32, and the reference's masked terms underflow to exactly 0.
"""

import sys

if "/opt/trn_rl_repo" not in sys.path:
    sys.path.insert(0, "/opt/trn_rl_repo")

from contextlib import ExitStack

import numpy as np
import ml_dtypes

import concourse.bass as bass
import concourse.tile as tile
from concourse import bacc, mybir
from concourse import bass_utils

B, S, D = 2, 2048, 2048
NH, NKV, HD = 16, 4, 128
G = NH // NKV  # q heads per core
N_CORES = 8
CH = 512  # sq chunk width
NCH = S // CH  # 4
DT = D // 128  # 16 d-tiles
ST = S // 128  # 16 s-tiles
SCALE = 1.0 / float(np.sqrt(HD))
ROPE_BASE = 10000.0
BF16 = mybir.dt.bfloat16
F32 = mybir.dt.float32


def build_kernel():
    nc = bacc.Bacc("TRN2", target_bir_lowering=False, debug=False, num_devices=N_CORES)
    xT = nc.dram_tensor("xT", [D, S], BF16, kind="ExternalInput").ap()
    wq = nc.dram_tensor("wq", [D, G * HD], BF16, kind="ExternalInput").ap()
    wk = nc.dram_tensor("wk", [D, HD], BF16, kind="ExternalInput").ap()
    wv = nc.dram_tensor("wv", [D, HD], BF16, kind="ExternalInput").ap()
    wo = nc.dram_tensor("wo", [G * HD, D], BF16, kind="ExternalInput").ap()
    csT = nc.dram_tensor("csT", [128, S], F32, kind="ExternalInput").ap()
    tri = nc.dram_tensor("tri", [128, 128], BF16, kind="ExternalInput").ap()
    out = nc.dram_tensor("out", [S, D], BF16, kind="ExternalOutput").ap()

    with tile.TileContext(nc) as tc, ExitStack() as ctx:
        consts = ctx.enter_context(tc.tile_pool(name="consts", bufs=1))
        work = ctx.enter_context(tc.tile_pool(name="work", bufs=1))
        ps = ctx.enter_context(tc.tile_pool(name="ps", bufs=8, space="PSUM"))

        wq_s = consts.tile([128, DT, G * HD], BF16)
        wk_s = consts.tile([128, DT, HD], BF16)
        wv_s = consts.tile([128, DT, HD], BF16)
        wo_s = consts.tile([128, G, D], BF16)
        cs_s = consts.tile([128, S], F32)  # cos rows 0-63, sin rows 64-127
        tri_s = consts.tile([128, 128], BF16)
        iden_s = consts.tile([128, 128], BF16)
        from concourse.masks import make_identity

        make_identity(nc, iden_s[:])
        ones_s = nc.const_aps.aps[(BF16, 1.0)]  # [128, 1] of 1.0

        qt = work.tile([128, G, S], BF16)  # Q^T per head, rope'd
        kt = work.tile([128, S], BF16)  # K^T, rope'd
        vt = work.tile([128, S], BF16)  # V^T
        v = work.tile([128, ST, HD], BF16)  # V natural [s, d] tiles
        att = work.tile([128, G, S], BF16)  # attn_out^T per head

        # ---- fused per-chunk pipeline ----
        # For each 512-wide sq chunk c: stream x^T chunk, project q/k/v for
        # that chunk (one rolling PSUM bank per r-block), transpose the V
        # s-tiles, run the chunk's attention (sk-pipelined scores+exp vs
        # AV/denominator matmuls), normalize, and do the chunk's slice of the
        # output projection. Later chunks' DMA/projection overlaps earlier
        # chunks' attention.
        xsp = ctx.enter_context(tc.tile_pool(name="xsp", bufs=3))
        ropet = ctx.enter_context(tc.tile_pool(name="ropet", bufs=4))
        ehp = ctx.enter_context(tc.tile_pool(name="ehp", bufs=12))
        nrm = ctx.enter_context(tc.tile_pool(name="nrm", bufs=3))
        otp = ctx.enter_context(tc.tile_pool(name="otp", bufs=2))

        xT_r = xT.rearrange("(dt p) s -> p dt s", p=128)
        wq_r = wq.rearrange("(dt p) n -> p dt n", p=128)
        wk_r = wk.rearrange("(dt p) n -> p dt n", p=128)
        wv_r = wv.rearrange("(dt p) n -> p dt n", p=128)
        # tri early (tiny); cos/sin stream per chunk; Wo slices stream on the
        # ACT queue, one per chunk (needed first at the chunk-0 Wo stage)
        nc.sync.dma_start(tri_s[:], tri[:])
        wo_r = wo.rearrange("(ci p) n -> p ci n", p=128)

        def rope_block(dest_even, dest_odd, pst, c):
            csl = cs_s[0:64, c * CH : (c + 1) * CH]
            ssl = cs_s[64:128, c * CH : (c + 1) * CH]
            qe = pst[0:64, :]
            qo = pst[64:128, :]
            t3 = ropet.tile([64, CH], F32, tag="tc")
            nc.vector.tensor_mul(t3[:], qe, ssl)
            t4 = ropet.tile([64, CH], F32, tag="td")
            nc.vector.tensor_mul(t4[:], qo, csl)
            t1 = ropet.tile([64, CH], F32, tag="ta")
            nc.vector.tensor_mul(t1[:], qe, csl)
            t2 = ropet.tile([64, CH], F32, tag="tb")
            nc.vector.tensor_mul(t2[:], qo, ssl)
            nc.vector.tensor_sub(dest_even, t1[:], t2[:])
            nc.vector.tensor_add(dest_odd, t3[:], t4[:])

        LAG = 3

        xs_tiles = {}

        def emit_xs(c):
            if c == 1 and 1 in xs_tiles:
                csl = slice(CH, 2 * CH)
                nc.sync.dma_start(cs_s[:, csl], csT[:, csl])
                return
            csl = slice(c * CH, (c + 1) * CH)
            xsc = xsp.tile([128, DT, CH], BF16, tag="xs", name=f"xs_{c}")
            nc.sync.dma_start(cs_s[:, csl], csT[:, csl])
            if c == 0:
                # chunk 0: small first bites so the k-projection starts ASAP,
                # with chunk-1 x^T quarters interleaved into the stream so
                # the chunk-1 projection is never starved
                csl1 = slice(CH, 2 * CH)
                xsc1 = xsp.tile([128, DT, CH], BF16, tag="xs", name="xs_1")
                nc.sync.dma_start(wk_s[:, 0:4, :], wk_r[:, 0:4, :])
                nc.sync.dma_start(xsc[:, 0, :], xT_r[:, 0, csl])
                nc.sync.dma_start(wk_s[:, 4:16, :], wk_r[:, 4:16, :])
                for d in range(1, DT):
                    nc.sync.dma_start(xsc[:, d, :], xT_r[:, d, csl])
                    nc.sync.dma_start(wq_s[:, d - 1, :], wq_r[:, d - 1, :])
                    if d % 4 == 0:
                        nc.sync.dma_start(
                            xsc1[:, d - 4 : d, :], xT_r[:, d - 4 : d, csl1]
                        )
                nc.sync.dma_start(wq_s[:, DT - 1, :], wq_r[:, DT - 1, :])
                nc.sync.dma_start(wv_s[:], wv_r[:])
                nc.sync.dma_start(xsc1[:, 12:16, :], xT_r[:, 12:16, csl1])
                xs_tiles[1] = xsc1
            elif c == 1:
                pass  # tile + DMAs already emitted with chunk 0
            else:
                nc.sync.dma_start(xsc[:, 0:8, :], xT_r[:, 0:8, csl])
                nc.sync.dma_start(xsc[:, 8:16, :], xT_r[:, 8:16, csl])
            if c == 0:
                # all four Wo row-blocks are needed by the first s-tile's
                # output projection; stream them on the ACT hwdge queue
                for ci in range(G):
                    nc.scalar.dma_start(wo_s[:, ci, :], wo_r[:, ci, :])
            xs_tiles[c] = xsc

        def emit_proj(c):
            csl = slice(c * CH, (c + 1) * CH)
            xsc = xs_tiles.pop(c)
            # projections for this chunk: k first (attention needs it
            # soonest), then q heads, then v
            for r in [G] + list(range(G)) + [G + 1]:
                if r < G:
                    w_view = wq_s[:, :, r * HD : (r + 1) * HD]
                elif r == G:
                    w_view = wk_s[:, :, :]
                else:
                    w_view = wv_s[:, :, :]
                pp = ps.tile([128, CH], F32, tag="ps", name=f"pp_{c}_{r}")
                for d in range(DT):
                    nc.tensor.matmul(
                        pp[:],
                        lhsT=w_view[:, d, :],
                        rhs=xsc[:, d, :],
                        start=(d == 0),
                        stop=(d == DT - 1),
                    )
                if r < G:
                    rope_block(qt[0:64, r, csl], qt[64:128, r, csl], pp[:], c)
                elif r == G:
                    rope_block(kt[0:64, csl], kt[64:128, csl], pp[:], c)
                else:
                    nc.vector.tensor_copy(vt[:, csl], pp[:])

        def emit_vtrans(c):
            # V^T -> V for this chunk's 4 s-tiles (PE transpose)
            for st in range(4 * c, 4 * c + 4):
                tp = ps.tile([128, 128], BF16, tag="ps", name=f"tp_{st}")
                nc.tensor.transpose(
                    tp[:], vt[:, st * 128 : (st + 1) * 128], iden_s[:]
                )
                nc.vector.tensor_copy(v[:, st, :], tp[:])

        # ---- attention + output projection over 128-wide sq tiles ----
        # For each (st, sk) causal pair: ONE score matmul (4 heads packed on
        # the moving side, N=512), one exp, one AV matmul (all heads into one
        # PSUM bank), one denominator matmul. av lags scores by LAG pairs so
        # the PE never waits on the exp of its own pair. After an s-tile's
        # last AV, its normalize + Wo output projection are emitted inline.
        state = {"avp": None, "denp": None}
        ehs = {}

        def emit_sc(st, sk):
            stsl = slice(st * 128, (st + 1) * 128)
            scp = ps.tile([128, CH], F32, tag="ps", name=f"sc_{st}_{sk}")
            nc.tensor.matmul(
                scp[:],
                lhsT=kt[:, sk * 128 : (sk + 1) * 128],
                rhs=qt[:, :, stsl],
                start=True,
                stop=True,
            )
            eh = ehp.tile([128, G, 128], BF16, tag="eh", name=f"eh_{st}_{sk}")
            nc.scalar.activation(
                eh[:].rearrange("p h s -> p (h s)"),
                scp[:],
                mybir.ActivationFunctionType.Exp,
                scale=SCALE,
            )
            if sk == st:
                nc.vector.tensor_mul(
                    eh[:],
                    eh[:],
                    tri_s[:, None, :].to_broadcast((128, G, 128)),
                )
            ehs[(st, sk)] = eh

        def emit_av(st, sk):
            if sk == 0:
                state["avp"] = ps.tile([128, CH], F32, tag="ps", name=f"av_{st}")
                state["denp"] = ps.tile([128, CH], F32, tag="ps", name=f"den_{st}")
            avp, denp = state["avp"], state["denp"]
            eh = ehs.pop((st, sk))
            ehf = eh[:].rearrange("p h s -> p (h s)")
            nc.tensor.matmul(
                avp[:],
                lhsT=v[:, sk, :],
                rhs=ehf,
                start=(sk == 0),
                stop=(sk == st),
            )
            nc.tensor.matmul(
                denp[0:1, :],
                lhsT=ones_s[:, 0:1],
                rhs=ehf,
                start=(sk == 0),
                stop=(sk == st),
            )
            if sk == st:
                emit_norm(st, avp, denp)
                wo_queue.append(st)
                if len(wo_queue) > 2:
                    emit_wo(wo_queue.pop(0))

        wo_queue = []

        def emit_norm(st, avp, denp):
            stsl = slice(st * 128, (st + 1) * 128)
            rec = nrm.tile([1, CH], F32, tag="rec")
            nc.vector.reciprocal(rec[:], denp[0:1, :])
            bc = nrm.tile([128, CH], F32, tag="bc")
            nc.gpsimd.partition_broadcast(bc[:], rec[:])
            nc.vector.tensor_mul(
                att[:, :, stsl],
                avp[:].rearrange("p (h s) -> p h s", h=G),
                bc[:].rearrange("p (h s) -> p h s", h=G),
            )

        def emit_wo(st):
            stsl = slice(st * 128, (st + 1) * 128)
            ot = otp.tile([128, S], BF16, tag="ot", name=f"ot_{st}")
            for n in range(NCH):
                po = ps.tile([128, CH], F32, tag="ps", name=f"po_{st}_{n}")
                for ci in range(G):
                    nc.tensor.matmul(
                        po[:],
                        lhsT=att[:, ci, stsl],
                        rhs=wo_s[:, ci, n * CH : (n + 1) * CH],
                        start=(ci == 0),
                        stop=(ci == G - 1),
                    )
                nc.scalar.copy(ot[:, n * CH : (n + 1) * CH], po[:])
            nc.sync.dma_start(out[st * 128 : (st + 1) * 128, :], ot[:])

        emit_xs(0)
        emit_proj(0)
        emit_vtrans(0)
        emit_xs(1)
        emit_proj(1)
        emit_xs(2)
        for c in range(NCH):
            pairs = [
                (st, sk) for st in range(4 * c, 4 * c + 4) for sk in range(st + 1)
            ]
            for i, p in enumerate(pairs):
                emit_sc(*p)
                if i >= LAG:
                    emit_av(*pairs[i - LAG])
            if c + 3 < NCH:
                emit_xs(c + 3)
            if c + 2 < NCH:
                emit_proj(c + 2)
            for p in pairs[len(pairs) - LAG :]:
                emit_av(*p)
            if c + 1 < NCH:
                emit_vtrans(c + 1)
        while wo_queue:
            emit_wo(wo_queue.pop(0))

    nc.compile()
    return nc


_EXEC_CACHE = None


def _get_exec():
    """Build the Bass program once and wrap it in a cached jitted shard_map.

    Inputs are concatenated on axis 0 across a (batch=2, group=4) device mesh
    so that batch-replicated tensors (weights, tables) and group-replicated
    tensors (x^T) are only transferred once each, and the jit/NEFF lowering
    happens a single time per process.
    """
    global _EXEC_CACHE
    if _EXEC_CACHE is not None:
        return _EXEC_CACHE

    import jax
    from jax.experimental.shard_map import shard_map
    from jax.sharding import Mesh, PartitionSpec

    from concourse import bass2jax, mybir as _mybir

    nc = build_kernel()
    bass2jax.install_neuronx_cc_hook()

    partition_name = (
        nc.partition_id_tensor.name if nc.partition_id_tensor is not None else None
    )
    in_names = []
    out_names = []
    out_avals = []
    for alloc in nc.m.functions[0].allocations:
        if not isinstance(alloc, _mybir.MemoryLocationSet):
            continue
        name = alloc.memorylocations[0].name
        if alloc.kind == "ExternalInput":
            if name != partition_name:
                in_names.append(name)
        elif alloc.kind == "ExternalOutput":
            out_names.append(name)
            out_avals.append(
                jax.core.ShapedArray(
                    tuple(alloc.tensor_shape), _mybir.dt.np(alloc.dtype)
                )
            )
    n_params = len(in_names)
    all_in_names = tuple(in_names) + tuple(out_names)
    if partition_name is not None:
        all_in_names = all_in_names + (partition_name,)

    def _body(*args):
        operands = list(args)
        if partition_name is not None:
            operands.append(bass2jax.partition_id_tensor())
        outs = bass2jax._bass_exec_p.bind(
            *operands,
            out_avals=tuple(out_avals),
            in_names=all_in_names,
            out_names=tuple(out_names),
            lowering_input_output_aliases=(),
            sim_require_finite=True,
            sim_require_nnan=True,
            nc=nc,
        )
        return tuple(outs)

    devices = jax.devices()[:N_CORES]
    mesh = Mesh(np.asarray(devices).reshape(B, NKV), ("b", "g"))
    # sharding of the axis-0-concatenated global inputs, in in_names order
    spec_by_name = {
        "xT": PartitionSpec("b"),  # [B*D, S]
        "wq": PartitionSpec("g"),  # [NKV*D, G*HD]
        "wk": PartitionSpec("g"),
        "wv": PartitionSpec("g"),
        "wo": PartitionSpec("g"),  # [NKV*G*HD, D]
        "csT": PartitionSpec(),  # replicated
        "tri": PartitionSpec(),
        "out": PartitionSpec(("b", "g")),
    }
    in_specs = tuple(spec_by_name[n] for n in tuple(in_names) + tuple(out_names))
    out_specs = tuple(spec_by_name[n] for n in out_names)
    donate = tuple(range(n_params, n_params + len(out_names)))
    sharded = jax.jit(
        shard_map(
            _body, mesh=mesh, in_specs=in_specs, out_specs=out_specs, check_rep=False
        ),
        donate_argnums=donate,
        keep_unused=True,
    )

    out_sharding = jax.sharding.NamedSharding(mesh, PartitionSpec(("b", "g")))
    zeros_fn = jax.jit(
        lambda: jax.numpy.zeros((N_CORES * S, D), jax.numpy.bfloat16),
        out_shardings=out_sharding,
    )

    _EXEC_CACHE = (sharded, tuple(in_names), mesh, zeros_fn)
    return _EXEC_CACHE


def _rope_tables():
    inv_freq = 1.0 / ROPE_BASE ** (np.arange(0, HD, 2, dtype=np.float32) / HD)
    t = np.arange(S, dtype=np.float32)
    freqs = np.outer(t, inv_freq)  # [S, HD/2]
    return (
        np.ascontiguousarray(np.cos(freqs).T.astype(np.float32)),
        np.ascontiguousarray(np.sin(freqs).T.astype(np.float32)),
    )


def make_global_inputs(x, Wq, Wk, Wv, Wo):
    """Axis-0-concatenated global arrays, keyed by DRAM tensor name."""
    bf16 = ml_dtypes.bfloat16
    # per-head evens-then-odds column permutation
    perm_h = np.concatenate([np.arange(0, HD, 2), np.arange(1, HD, 2)])
    perm_q = np.concatenate([h * HD + perm_h for h in range(NH)])
    perm_k = np.concatenate([h * HD + perm_h for h in range(NKV)])
    x = np.asarray(x)
    Wq_p = np.asarray(Wq)[:, perm_q].astype(bf16)
    Wk_p = np.asarray(Wk)[:, perm_k].astype(bf16)
    Wv = np.asarray(Wv).astype(bf16)
    cosT, sinT = _rope_tables()
    csT = np.concatenate([cosT, sinT], axis=0)  # [128, S]
    tri = (np.arange(128)[:, None] <= np.arange(128)[None, :]).astype(bf16)

    xT_g = np.concatenate([x[b].T for b in range(B)], axis=0).astype(bf16)
    wq_g = np.concatenate(
        [Wq_p[:, g * G * HD : (g + 1) * G * HD] for g in range(NKV)], axis=0
    )
    wk_g = np.concatenate([Wk_p[:, g * HD : (g + 1) * HD] for g in range(NKV)], axis=0)
    wv_g = np.concatenate([Wv[:, g * HD : (g + 1) * HD] for g in range(NKV)], axis=0)
    wo_g = np.asarray(Wo).astype(bf16)  # row-slice concat over g == Wo itself
    return {
        "xT": xT_g,
        "wq": wq_g,
        "wk": wk_g,
        "wv": wv_g,
        "wo": wo_g,
        "csT": csT,
        "tri": tri,
    }


def run_global(inputs_g, time_exec=False):
    """Run the kernel on pre-built global input arrays; returns [B, S, D] f32."""
    import jax
    import time as _time

    sharded, in_names, mesh, zeros_fn = _get_exec()
    args = [inputs_g[n] for n in in_names]
    out_g = sharded(*args, zeros_fn())
    if time_exec:
        # device_put inputs once, then time execution only
        from jax.sharding import NamedSharding

        dev_args = [
            jax.device_put(a, NamedSharding(mesh, s))
            for a, s in zip(args, sharded_in_specs())
        ]
        jax.block_until_ready(dev_args)
        times = []
        for _ in range(5):
            z = zeros_fn()
            jax.block_until_ready(z)
            t0 = _time.perf_counter()
            o = sharded(*dev_args, z)
            jax.block_until_ready(o)
            times.append(_time.perf_counter() - t0)
        print(f"exec-only times (ms): {[f'{t*1e3:.2f}' for t in times]}")
        out_g = o
    out = (
        np.asarray(out_g)
        .astype(np.float32)
        .reshape(B, NKV, S, D)
        .sum(axis=1, dtype=np.float32)
    )
    return out


def sharded_in_specs():
    from jax.sharding import PartitionSpec

    spec_by_name = {
        "xT": PartitionSpec("b"),
        "wq": PartitionSpec("g"),
        "wk": PartitionSpec("g"),
        "wv": PartitionSpec("g"),
        "wo": PartitionSpec("g"),
        "csT": PartitionSpec(),
        "tri": PartitionSpec(),
    }
    _, in_names, _, _ = _get_exec()
    return [spec_by_name[n] for n in in_names]


def kernel(x, mask, Wq, Wk, Wv, Wo):
    inputs_g = make_global_inputs(x, Wq, Wk, Wv, Wo)
    return run_global(inputs_g)


# revision 60
# speedup vs baseline: 1.0018x; 1.0018x over previous
"""Llama GQA attention (B=2, S=2048, D=2048, NH=16, NKV=4, HD=128) on 8 TRN2 cores.

Sharding: core c = batch*4 + kv_group  (2 batches x 4 kv groups).
Each core computes 4 q heads + 1 kv head for one batch, then a partial
output projection with its 512-row slice of Wo; the host sums the 4
partials per batch.

Device-side layout trick: everything is computed in "transposed"
orientation (QT/KT = [head_dim, S]) so that
  - projections take x^T tiles as the moving operand (host supplies x^T),
  - scoresT[sk, sq] = KT_tile^T @ QT needs no transposes,
  - softmax exp runs on scoresT, denominators come from ones^T @ expT
    matmuls (col-packed via tile_position), and
  - attn_outT[d, sq] accumulates with lhsT = V tile in natural [s, d]
    layout, rhs = expT; attn_outT then feeds the Wo matmul as lhsT.
Causal structure: score/AV/denominator matmuls are only emitted for
lower-triangular (sk, sq) tile pairs; the 128x128 diagonal blocks are
masked with a precomputed 0/1 triangle after exp.

Per-chunk schedule: all score matmuls + exps complete first (expT tiles
buffered in SBUF), then the AV + denominator matmuls stream back-to-back
on the PE with no ACT dependency, then the normalize (reciprocal ->
partition-broadcast -> multiply) runs while the next chunk's scores start.

RoPE: host permutes Wq/Wk columns within each head to evens-then-odds, so
the interleaved-pair rotation becomes contiguous half-rotations on the
partition dim; q.k dot products are invariant to the (shared) permutation.
Softmax max-subtraction is skipped: scores/sqrt(128) are O(10), exp is
safe in f32, and the reference's masked terms underflow to exactly 0.
"""

import sys

if "/opt/trn_rl_repo" not in sys.path:
    sys.path.insert(0, "/opt/trn_rl_repo")

from contextlib import ExitStack

import numpy as np
import ml_dtypes

import concourse.bass as bass
import concourse.tile as tile
from concourse import bacc, mybir
from concourse import bass_utils

B, S, D = 2, 2048, 2048
NH, NKV, HD = 16, 4, 128
G = NH // NKV  # q heads per core
N_CORES = 8
CH = 512  # sq chunk width
NCH = S // CH  # 4
DT = D // 128  # 16 d-tiles
ST = S // 128  # 16 s-tiles
SCALE = 1.0 / float(np.sqrt(HD))
ROPE_BASE = 10000.0
BF16 = mybir.dt.bfloat16
F32 = mybir.dt.float32


def build_kernel():
    nc = bacc.Bacc("TRN2", target_bir_lowering=False, debug=False, num_devices=N_CORES)
    xT = nc.dram_tensor("xT", [D, S], BF16, kind="ExternalInput").ap()
    wq = nc.dram_tensor("wq", [D, G * HD], BF16, kind="ExternalInput").ap()
    wk = nc.dram_tensor("wk", [D, HD], BF16, kind="ExternalInput").ap()
    wv = nc.dram_tensor("wv", [D, HD], BF16, kind="ExternalInput").ap()
    wo = nc.dram_tensor("wo", [G * HD, D], BF16, kind="ExternalInput").ap()
    csT = nc.dram_tensor("csT", [128, S], F32, kind="ExternalInput").ap()
    tri = nc.dram_tensor("tri", [128, 128], BF16, kind="ExternalInput").ap()
    out = nc.dram_tensor("out", [S, D], BF16, kind="ExternalOutput").ap()

    with tile.TileContext(nc) as tc, ExitStack() as ctx:
        consts = ctx.enter_context(tc.tile_pool(name="consts", bufs=1))
        work = ctx.enter_context(tc.tile_pool(name="work", bufs=1))
        ps = ctx.enter_context(tc.tile_pool(name="ps", bufs=8, space="PSUM"))

        wq_s = consts.tile([128, DT, G * HD], BF16)
        wk_s = consts.tile([128, DT, HD], BF16)
        wv_s = consts.tile([128, DT, HD], BF16)
        wo_s = consts.tile([128, G, D], BF16)
        cs_s = consts.tile([128, S], F32)  # cos rows 0-63, sin rows 64-127
        tri_s = consts.tile([128, 128], BF16)
        iden_s = consts.tile([128, 128], BF16)
        from concourse.masks import make_identity

        make_identity(nc, iden_s[:])
        ones_s = nc.const_aps.aps[(BF16, 1.0)]  # [128, 1] of 1.0

        qt = work.tile([128, G, S], BF16)  # Q^T per head, rope'd
        kt = work.tile([128, S], BF16)  # K^T, rope'd
        vt = work.tile([128, S], BF16)  # V^T
        v = work.tile([128, ST, HD], BF16)  # V natural [s, d] tiles
        att = work.tile([128, G, S], BF16)  # attn_out^T per head

        # ---- fused per-chunk pipeline ----
        # For each 512-wide sq chunk c: stream x^T chunk, project q/k/v for
        # that chunk (one rolling PSUM bank per r-block), transpose the V
        # s-tiles, run the chunk's attention (sk-pipelined scores+exp vs
        # AV/denominator matmuls), normalize, and do the chunk's slice of the
        # output projection. Later chunks' DMA/projection overlaps earlier
        # chunks' attention.
        xsp = ctx.enter_context(tc.tile_pool(name="xsp", bufs=3))
        ropet = ctx.enter_context(tc.tile_pool(name="ropet", bufs=4))
        ehp = ctx.enter_context(tc.tile_pool(name="ehp", bufs=12))
        nrm = ctx.enter_context(tc.tile_pool(name="nrm", bufs=3))
        otp = ctx.enter_context(tc.tile_pool(name="otp", bufs=2))

        xT_r = xT.rearrange("(dt p) s -> p dt s", p=128)
        wq_r = wq.rearrange("(dt p) n -> p dt n", p=128)
        wk_r = wk.rearrange("(dt p) n -> p dt n", p=128)
        wv_r = wv.rearrange("(dt p) n -> p dt n", p=128)
        # tri early (tiny); cos/sin stream per chunk; Wo slices stream on the
        # ACT queue, one per chunk (needed first at the chunk-0 Wo stage)
        nc.sync.dma_start(tri_s[:], tri[:])
        wo_r = wo.rearrange("(ci p) n -> p ci n", p=128)

        def rope_block(dest_even, dest_odd, pst, c):
            csl = cs_s[0:64, c * CH : (c + 1) * CH]
            ssl = cs_s[64:128, c * CH : (c + 1) * CH]
            qe = pst[0:64, :]
            qo = pst[64:128, :]
            t3 = ropet.tile([64, CH], F32, tag="tc")
            nc.vector.tensor_mul(t3[:], qe, ssl)
            t4 = ropet.tile([64, CH], F32, tag="td")
            nc.vector.tensor_mul(t4[:], qo, csl)
            t1 = ropet.tile([64, CH], F32, tag="ta")
            nc.vector.tensor_mul(t1[:], qe, csl)
            t2 = ropet.tile([64, CH], F32, tag="tb")
            nc.vector.tensor_mul(t2[:], qo, ssl)
            nc.vector.tensor_sub(dest_even, t1[:], t2[:])
            nc.vector.tensor_add(dest_odd, t3[:], t4[:])

        LAG = 5

        xs_tiles = {}

        def emit_xs(c):
            if c == 1 and 1 in xs_tiles:
                csl = slice(CH, 2 * CH)
                nc.sync.dma_start(cs_s[:, csl], csT[:, csl])
                return
            csl = slice(c * CH, (c + 1) * CH)
            xsc = xsp.tile([128, DT, CH], BF16, tag="xs", name=f"xs_{c}")
            nc.sync.dma_start(cs_s[:, csl], csT[:, csl])
            if c == 0:
                # chunk 0: small first bites so the k-projection starts ASAP,
                # with chunk-1 x^T quarters interleaved into the stream so
                # the chunk-1 projection is never starved
                csl1 = slice(CH, 2 * CH)
                xsc1 = xsp.tile([128, DT, CH], BF16, tag="xs", name="xs_1")
                nc.sync.dma_start(wk_s[:, 0:4, :], wk_r[:, 0:4, :])
                nc.sync.dma_start(xsc[:, 0, :], xT_r[:, 0, csl])
                nc.sync.dma_start(wk_s[:, 4:16, :], wk_r[:, 4:16, :])
                for d in range(1, DT):
                    nc.sync.dma_start(xsc[:, d, :], xT_r[:, d, csl])
                    nc.sync.dma_start(wq_s[:, d - 1, :], wq_r[:, d - 1, :])
                    if d % 4 == 0:
                        nc.sync.dma_start(
                            xsc1[:, d - 4 : d, :], xT_r[:, d - 4 : d, csl1]
                        )
                nc.sync.dma_start(wq_s[:, DT - 1, :], wq_r[:, DT - 1, :])
                nc.sync.dma_start(wv_s[:], wv_r[:])
                nc.sync.dma_start(xsc1[:, 12:16, :], xT_r[:, 12:16, csl1])
                xs_tiles[1] = xsc1
            elif c == 1:
                pass  # tile + DMAs already emitted with chunk 0
            else:
                nc.sync.dma_start(xsc[:, 0:8, :], xT_r[:, 0:8, csl])
                nc.sync.dma_start(xsc[:, 8:16, :], xT_r[:, 8:16, csl])
            if c == 0:
                # all four Wo row-blocks are needed by the first s-tile's
                # output projection; stream them on the ACT hwdge queue
                for ci in range(G):
                    nc.scalar.dma_start(wo_s[:, ci, :], wo_r[:, ci, :])
            xs_tiles[c] = xsc

        def emit_proj(c):
            csl = slice(c * CH, (c + 1) * CH)
            xsc = xs_tiles.pop(c)
            # projections for this chunk: k first (attention needs it
            # soonest), then q heads, then v
            for r in [G] + list(range(G)) + [G + 1]:
                if r < G:
                    w_view = wq_s[:, :, r * HD : (r + 1) * HD]
                elif r == G:
                    w_view = wk_s[:, :, :]
                else:
                    w_view = wv_s[:, :, :]
                pp = ps.tile([128, CH], F32, tag="ps", name=f"pp_{c}_{r}")
                for d in range(DT):
                    nc.tensor.matmul(
                        pp[:],
                        lhsT=w_view[:, d, :],
                        rhs=xsc[:, d, :],
                        start=(d == 0),
                        stop=(d == DT - 1),
                    )
                if r < G:
                    rope_block(qt[0:64, r, csl], qt[64:128, r, csl], pp[:], c)
                elif r == G:
                    rope_block(kt[0:64, csl], kt[64:128, csl], pp[:], c)
                else:
                    nc.vector.tensor_copy(vt[:, csl], pp[:])

        def emit_vtrans(c):
            # V^T -> V for this chunk's 4 s-tiles (PE transpose)
            for st in range(4 * c, 4 * c + 4):
                tp = ps.tile([128, 128], BF16, tag="ps", name=f"tp_{st}")
                nc.tensor.transpose(
                    tp[:], vt[:, st * 128 : (st + 1) * 128], iden_s[:]
                )
                nc.vector.tensor_copy(v[:, st, :], tp[:])

        # ---- attention + output projection over 128-wide sq tiles ----
        # For each (st, sk) causal pair: ONE score matmul (4 heads packed on
        # the moving side, N=512), one exp, one AV matmul (all heads into one
        # PSUM bank), one denominator matmul. av lags scores by LAG pairs so
        # the PE never waits on the exp of its own pair. After an s-tile's
        # last AV, its normalize + Wo output projection are emitted inline.
        state = {"avp": None, "denp": None}
        ehs = {}

        def emit_sc(st, sk):
            stsl = slice(st * 128, (st + 1) * 128)
            scp = ps.tile([128, CH], F32, tag="ps", name=f"sc_{st}_{sk}")
            nc.tensor.matmul(
                scp[:],
                lhsT=kt[:, sk * 128 : (sk + 1) * 128],
                rhs=qt[:, :, stsl],
                start=True,
                stop=True,
            )
            eh = ehp.tile([128, G, 128], BF16, tag="eh", name=f"eh_{st}_{sk}")
            nc.scalar.activation(
                eh[:].rearrange("p h s -> p (h s)"),
                scp[:],
                mybir.ActivationFunctionType.Exp,
                scale=SCALE,
            )
            if sk == st:
                nc.vector.tensor_mul(
                    eh[:],
                    eh[:],
                    tri_s[:, None, :].to_broadcast((128, G, 128)),
                )
            ehs[(st, sk)] = eh

        def emit_av(st, sk):
            if sk == 0:
                state["avp"] = ps.tile([128, CH], F32, tag="ps", name=f"av_{st}")
                state["denp"] = ps.tile([128, CH], F32, tag="ps", name=f"den_{st}")
            avp, denp = state["avp"], state["denp"]
            eh = ehs.pop((st, sk))
            ehf = eh[:].rearrange("p h s -> p (h s)")
            nc.tensor.matmul(
                avp[:],
                lhsT=v[:, sk, :],
                rhs=ehf,
                start=(sk == 0),
                stop=(sk == st),
            )
            nc.tensor.matmul(
                denp[0:1, :],
                lhsT=ones_s[:, 0:1],
                rhs=ehf,
                start=(sk == 0),
                stop=(sk == st),
            )
            if sk == st:
                emit_norm(st, avp, denp)
                wo_queue.append(st)
                if len(wo_queue) > 2:
                    emit_wo(wo_queue.pop(0))

        wo_queue = []

        def emit_norm(st, avp, denp):
            stsl = slice(st * 128, (st + 1) * 128)
            rec = nrm.tile([1, CH], F32, tag="rec")
            nc.vector.reciprocal(rec[:], denp[0:1, :])
            bc = nrm.tile([128, CH], F32, tag="bc")
            nc.gpsimd.partition_broadcast(bc[:], rec[:])
            nc.vector.tensor_mul(
                att[:, :, stsl],
                avp[:].rearrange("p (h s) -> p h s", h=G),
                bc[:].rearrange("p (h s) -> p h s", h=G),
            )

        def emit_wo(st):
            stsl = slice(st * 128, (st + 1) * 128)
            ot = otp.tile([128, S], BF16, tag="ot", name=f"ot_{st}")
            for n in range(NCH):
                po = ps.tile([128, CH], F32, tag="ps", name=f"po_{st}_{n}")
                for ci in range(G):
                    nc.tensor.matmul(
                        po[:],
                        lhsT=att[:, ci, stsl],
                        rhs=wo_s[:, ci, n * CH : (n + 1) * CH],
                        start=(ci == 0),
                        stop=(ci == G - 1),
                    )
                nc.scalar.copy(ot[:, n * CH : (n + 1) * CH], po[:])
            nc.sync.dma_start(out[st * 128 : (st + 1) * 128, :], ot[:])

        emit_xs(0)
        emit_proj(0)
        emit_vtrans(0)
        emit_xs(1)
        emit_proj(1)
        emit_xs(2)
        for c in range(NCH):
            pairs = [
                (st, sk) for st in range(4 * c, 4 * c + 4) for sk in range(st + 1)
            ]
            for i, p in enumerate(pairs):
                emit_sc(*p)
                if i >= LAG:
                    emit_av(*pairs[i - LAG])
            if c + 3 < NCH:
                emit_xs(c + 3)
            if c + 2 < NCH:
                emit_proj(c + 2)
            for p in pairs[len(pairs) - LAG :]:
                emit_av(*p)
            if c + 1 < NCH:
                emit_vtrans(c + 1)
            # drain deferred output projections now: an in-order PE can't
            # reach them past the next chunk's first (rope-blocked) scores,
            # and they are exactly the work that hides the rope chain
            while wo_queue:
                emit_wo(wo_queue.pop(0))

    nc.compile()
    return nc


_EXEC_CACHE = None


def _get_exec():
    """Build the Bass program once and wrap it in a cached jitted shard_map.

    Inputs are concatenated on axis 0 across a (batch=2, group=4) device mesh
    so that batch-replicated tensors (weights, tables) and group-replicated
    tensors (x^T) are only transferred once each, and the jit/NEFF lowering
    happens a single time per process.
    """
    global _EXEC_CACHE
    if _EXEC_CACHE is not None:
        return _EXEC_CACHE

    import jax
    from jax.experimental.shard_map import shard_map
    from jax.sharding import Mesh, PartitionSpec

    from concourse import bass2jax, mybir as _mybir

    nc = build_kernel()
    bass2jax.install_neuronx_cc_hook()

    partition_name = (
        nc.partition_id_tensor.name if nc.partition_id_tensor is not None else None
    )
    in_names = []
    out_names = []
    out_avals = []
    for alloc in nc.m.functions[0].allocations:
        if not isinstance(alloc, _mybir.MemoryLocationSet):
            continue
        name = alloc.memorylocations[0].name
        if alloc.kind == "ExternalInput":
            if name != partition_name:
                in_names.append(name)
        elif alloc.kind == "ExternalOutput":
            out_names.append(name)
            out_avals.append(
                jax.core.ShapedArray(
                    tuple(alloc.tensor_shape), _mybir.dt.np(alloc.dtype)
                )
            )
    n_params = len(in_names)
    all_in_names = tuple(in_names) + tuple(out_names)
    if partition_name is not None:
        all_in_names = all_in_names + (partition_name,)

    def _body(*args):
        operands = list(args)
        if partition_name is not None:
            operands.append(bass2jax.partition_id_tensor())
        outs = bass2jax._bass_exec_p.bind(
            *operands,
            out_avals=tuple(out_avals),
            in_names=all_in_names,
            out_names=tuple(out_names),
            lowering_input_output_aliases=(),
            sim_require_finite=True,
            sim_require_nnan=True,
            nc=nc,
        )
        return tuple(outs)

    devices = jax.devices()[:N_CORES]
    mesh = Mesh(np.asarray(devices).reshape(B, NKV), ("b", "g"))
    # sharding of the axis-0-concatenated global inputs, in in_names order
    spec_by_name = {
        "xT": PartitionSpec("b"),  # [B*D, S]
        "wq": PartitionSpec("g"),  # [NKV*D, G*HD]
        "wk": PartitionSpec("g"),
        "wv": PartitionSpec("g"),
        "wo": PartitionSpec("g"),  # [NKV*G*HD, D]
        "csT": PartitionSpec(),  # replicated
        "tri": PartitionSpec(),
        "out": PartitionSpec(("b", "g")),
    }
    in_specs = tuple(spec_by_name[n] for n in tuple(in_names) + tuple(out_names))
    out_specs = tuple(spec_by_name[n] for n in out_names)
    donate = tuple(range(n_params, n_params + len(out_names)))
    sharded = jax.jit(
        shard_map(
            _body, mesh=mesh, in_specs=in_specs, out_specs=out_specs, check_rep=False
        ),
        donate_argnums=donate,
        keep_unused=True,
    )

    out_sharding = jax.sharding.NamedSharding(mesh, PartitionSpec(("b", "g")))
    zeros_fn = jax.jit(
        lambda: jax.numpy.zeros((N_CORES * S, D), jax.numpy.bfloat16),
        out_shardings=out_sharding,
    )

    _EXEC_CACHE = (sharded, tuple(in_names), mesh, zeros_fn)
    return _EXEC_CACHE


def _rope_tables():
    inv_freq = 1.0 / ROPE_BASE ** (np.arange(0, HD, 2, dtype=np.float32) / HD)
    t = np.arange(S, dtype=np.float32)
    freqs = np.outer(t, inv_freq)  # [S, HD/2]
    return (
        np.ascontiguousarray(np.cos(freqs).T.astype(np.float32)),
        np.ascontiguousarray(np.sin(freqs).T.astype(np.float32)),
    )


def make_global_inputs(x, Wq, Wk, Wv, Wo):
    """Axis-0-concatenated global arrays, keyed by DRAM tensor name."""
    bf16 = ml_dtypes.bfloat16
    # per-head evens-then-odds column permutation
    perm_h = np.concatenate([np.arange(0, HD, 2), np.arange(1, HD, 2)])
    perm_q = np.concatenate([h * HD + perm_h for h in range(NH)])
    perm_k = np.concatenate([h * HD + perm_h for h in range(NKV)])
    x = np.asarray(x)
    Wq_p = np.asarray(Wq)[:, perm_q].astype(bf16)
    Wk_p = np.asarray(Wk)[:, perm_k].astype(bf16)
    Wv = np.asarray(Wv).astype(bf16)
    cosT, sinT = _rope_tables()
    csT = np.concatenate([cosT, sinT], axis=0)  # [128, S]
    tri = (np.arange(128)[:, None] <= np.arange(128)[None, :]).astype(bf16)

    xT_g = np.concatenate([x[b].T for b in range(B)], axis=0).astype(bf16)
    wq_g = np.concatenate(
        [Wq_p[:, g * G * HD : (g + 1) * G * HD] for g in range(NKV)], axis=0
    )
    wk_g = np.concatenate([Wk_p[:, g * HD : (g + 1) * HD] for g in range(NKV)], axis=0)
    wv_g = np.concatenate([Wv[:, g * HD : (g + 1) * HD] for g in range(NKV)], axis=0)
    wo_g = np.asarray(Wo).astype(bf16)  # row-slice concat over g == Wo itself
    return {
        "xT": xT_g,
        "wq": wq_g,
        "wk": wk_g,
        "wv": wv_g,
        "wo": wo_g,
        "csT": csT,
        "tri": tri,
    }


def run_global(inputs_g, time_exec=False):
    """Run the kernel on pre-built global input arrays; returns [B, S, D] f32."""
    import jax
    import time as _time

    sharded, in_names, mesh, zeros_fn = _get_exec()
    args = [inputs_g[n] for n in in_names]
    out_g = sharded(*args, zeros_fn())
    if time_exec:
        # device_put inputs once, then time execution only
        from jax.sharding import NamedSharding

        dev_args = [
            jax.device_put(a, NamedSharding(mesh, s))
            for a, s in zip(args, sharded_in_specs())
        ]
        jax.block_until_ready(dev_args)
        times = []
        for _ in range(5):
            z = zeros_fn()
            jax.block_until_ready(z)
            t0 = _time.perf_counter()
            o = sharded(*dev_args, z)
            jax.block_until_ready(o)
            times.append(_time.perf_counter() - t0)
        print(f"exec-only times (ms): {[f'{t*1e3:.2f}' for t in times]}")
        out_g = o
    out = (
        np.asarray(out_g)
        .astype(np.float32)
        .reshape(B, NKV, S, D)
        .sum(axis=1, dtype=np.float32)
    )
    return out


def sharded_in_specs():
    from jax.sharding import PartitionSpec

    spec_by_name = {
        "xT": PartitionSpec("b"),
        "wq": PartitionSpec("g"),
        "wk": PartitionSpec("g"),
        "wv": PartitionSpec("g"),
        "wo": PartitionSpec("g"),
        "csT": PartitionSpec(),
        "tri": PartitionSpec(),
    }
    _, in_names, _, _ = _get_exec()
    return [spec_by_name[n] for n in in_names]


def kernel(x, mask, Wq, Wk, Wv, Wo):
    inputs_g = make_global_inputs(x, Wq, Wk, Wv, Wo)
    return run_global(inputs_g)


# revision 61
# speedup vs baseline: 1.0057x; 1.0039x over previous
"""Llama GQA attention (B=2, S=2048, D=2048, NH=16, NKV=4, HD=128) on 8 TRN2 cores.

Sharding: core c = batch*4 + kv_group  (2 batches x 4 kv groups).
Each core computes 4 q heads + 1 kv head for one batch, then a partial
output projection with its 512-row slice of Wo; the host sums the 4
partials per batch.

Device-side layout trick: everything is computed in "transposed"
orientation (QT/KT = [head_dim, S]) so that
  - projections take x^T tiles as the moving operand (host supplies x^T),
  - scoresT[sk, sq] = KT_tile^T @ QT needs no transposes,
  - softmax exp runs on scoresT, denominators come from ones^T @ expT
    matmuls (col-packed via tile_position), and
  - attn_outT[d, sq] accumulates with lhsT = V tile in natural [s, d]
    layout, rhs = expT; attn_outT then feeds the Wo matmul as lhsT.
Causal structure: score/AV/denominator matmuls are only emitted for
lower-triangular (sk, sq) tile pairs; the 128x128 diagonal blocks are
masked with a precomputed 0/1 triangle after exp.

Per-chunk schedule: all score matmuls + exps complete first (expT tiles
buffered in SBUF), then the AV + denominator matmuls stream back-to-back
on the PE with no ACT dependency, then the normalize (reciprocal ->
partition-broadcast -> multiply) runs while the next chunk's scores start.

RoPE: host permutes Wq/Wk columns within each head to evens-then-odds, so
the interleaved-pair rotation becomes contiguous half-rotations on the
partition dim; q.k dot products are invariant to the (shared) permutation.
Softmax max-subtraction is skipped: scores/sqrt(128) are O(10), exp is
safe in f32, and the reference's masked terms underflow to exactly 0.
"""

import sys

if "/opt/trn_rl_repo" not in sys.path:
    sys.path.insert(0, "/opt/trn_rl_repo")

from contextlib import ExitStack

import numpy as np
import ml_dtypes

import concourse.bass as bass
import concourse.tile as tile
from concourse import bacc, mybir
from concourse import bass_utils

B, S, D = 2, 2048, 2048
NH, NKV, HD = 16, 4, 128
G = NH // NKV  # q heads per core
N_CORES = 8
CH = 512  # sq chunk width
NCH = S // CH  # 4
DT = D // 128  # 16 d-tiles
ST = S // 128  # 16 s-tiles
SCALE = 1.0 / float(np.sqrt(HD))
ROPE_BASE = 10000.0
BF16 = mybir.dt.bfloat16
F32 = mybir.dt.float32


def build_kernel():
    nc = bacc.Bacc("TRN2", target_bir_lowering=False, debug=False, num_devices=N_CORES)
    xT = nc.dram_tensor("xT", [D, S], BF16, kind="ExternalInput").ap()
    wq = nc.dram_tensor("wq", [D, G * HD], BF16, kind="ExternalInput").ap()
    wk = nc.dram_tensor("wk", [D, HD], BF16, kind="ExternalInput").ap()
    wv = nc.dram_tensor("wv", [D, HD], BF16, kind="ExternalInput").ap()
    wo = nc.dram_tensor("wo", [G * HD, D], BF16, kind="ExternalInput").ap()
    csT = nc.dram_tensor("csT", [128, S], F32, kind="ExternalInput").ap()
    tri = nc.dram_tensor("tri", [128, 128], BF16, kind="ExternalInput").ap()
    out = nc.dram_tensor("out", [S, D], BF16, kind="ExternalOutput").ap()

    with tile.TileContext(nc) as tc, ExitStack() as ctx:
        consts = ctx.enter_context(tc.tile_pool(name="consts", bufs=1))
        work = ctx.enter_context(tc.tile_pool(name="work", bufs=1))
        ps = ctx.enter_context(tc.tile_pool(name="ps", bufs=8, space="PSUM"))

        wq_s = consts.tile([128, DT, G * HD], BF16)
        wk_s = consts.tile([128, DT, HD], BF16)
        wv_s = consts.tile([128, DT, HD], BF16)
        wo_s = consts.tile([128, G, D], BF16)
        cs_s = consts.tile([128, S], F32)  # cos rows 0-63, sin rows 64-127
        tri_s = consts.tile([128, 128], BF16)
        iden_s = consts.tile([128, 128], BF16)
        from concourse.masks import make_identity

        make_identity(nc, iden_s[:])
        ones_s = nc.const_aps.aps[(BF16, 1.0)]  # [128, 1] of 1.0

        qt = work.tile([128, G, S], BF16)  # Q^T per head, rope'd
        kt = work.tile([128, S], BF16)  # K^T, rope'd
        vt = work.tile([128, S], BF16)  # V^T
        v = work.tile([128, ST, HD], BF16)  # V natural [s, d] tiles
        att = work.tile([128, G, S], BF16)  # attn_out^T per head

        # ---- fused per-chunk pipeline ----
        # For each 512-wide sq chunk c: stream x^T chunk, project q/k/v for
        # that chunk (one rolling PSUM bank per r-block), transpose the V
        # s-tiles, run the chunk's attention (sk-pipelined scores+exp vs
        # AV/denominator matmuls), normalize, and do the chunk's slice of the
        # output projection. Later chunks' DMA/projection overlaps earlier
        # chunks' attention.
        xsp = ctx.enter_context(tc.tile_pool(name="xsp", bufs=3))
        ropet = ctx.enter_context(tc.tile_pool(name="ropet", bufs=4))
        ehp = ctx.enter_context(tc.tile_pool(name="ehp", bufs=12))
        nrm = ctx.enter_context(tc.tile_pool(name="nrm", bufs=3))
        otp = ctx.enter_context(tc.tile_pool(name="otp", bufs=2))

        xT_r = xT.rearrange("(dt p) s -> p dt s", p=128)
        wq_r = wq.rearrange("(dt p) n -> p dt n", p=128)
        wk_r = wk.rearrange("(dt p) n -> p dt n", p=128)
        wv_r = wv.rearrange("(dt p) n -> p dt n", p=128)
        # tri early (tiny); cos/sin stream per chunk; Wo slices stream on the
        # ACT queue, one per chunk (needed first at the chunk-0 Wo stage)
        nc.scalar.dma_start(tri_s[:], tri[:])
        wo_r = wo.rearrange("(ci p) n -> p ci n", p=128)

        def rope_block(dest_even, dest_odd, pst, c):
            csl = cs_s[0:64, c * CH : (c + 1) * CH]
            ssl = cs_s[64:128, c * CH : (c + 1) * CH]
            qe = pst[0:64, :]
            qo = pst[64:128, :]
            t3 = ropet.tile([64, CH], F32, tag="tc")
            nc.vector.tensor_mul(t3[:], qe, ssl)
            t4 = ropet.tile([64, CH], F32, tag="td")
            nc.vector.tensor_mul(t4[:], qo, csl)
            t1 = ropet.tile([64, CH], F32, tag="ta")
            nc.vector.tensor_mul(t1[:], qe, csl)
            t2 = ropet.tile([64, CH], F32, tag="tb")
            nc.vector.tensor_mul(t2[:], qo, ssl)
            nc.vector.tensor_sub(dest_even, t1[:], t2[:])
            nc.vector.tensor_add(dest_odd, t3[:], t4[:])

        LAG = 5

        xs_tiles = {}

        def emit_xs(c):
            if c == 1 and 1 in xs_tiles:
                csl = slice(CH, 2 * CH)
                nc.sync.dma_start(cs_s[:, csl], csT[:, csl])
                return
            csl = slice(c * CH, (c + 1) * CH)
            xsc = xsp.tile([128, DT, CH], BF16, tag="xs", name=f"xs_{c}")
            if c != 0:
                nc.sync.dma_start(cs_s[:, csl], csT[:, csl])
            if c == 0:
                # chunk 0: small first bites so the k-projection starts ASAP,
                # with chunk-1 x^T quarters interleaved into the stream so
                # the chunk-1 projection is never starved
                csl1 = slice(CH, 2 * CH)
                xsc1 = xsp.tile([128, DT, CH], BF16, tag="xs", name="xs_1")
                nc.sync.dma_start(wk_s[:, 0:4, :], wk_r[:, 0:4, :])
                nc.sync.dma_start(xsc[:, 0, :], xT_r[:, 0, csl])
                nc.sync.dma_start(cs_s[:, csl], csT[:, csl])
                nc.sync.dma_start(wk_s[:, 4:16, :], wk_r[:, 4:16, :])
                for d in range(1, DT):
                    nc.sync.dma_start(xsc[:, d, :], xT_r[:, d, csl])
                    nc.sync.dma_start(wq_s[:, d - 1, :], wq_r[:, d - 1, :])
                    if d % 4 == 0:
                        nc.sync.dma_start(
                            xsc1[:, d - 4 : d, :], xT_r[:, d - 4 : d, csl1]
                        )
                nc.sync.dma_start(wq_s[:, DT - 1, :], wq_r[:, DT - 1, :])
                nc.sync.dma_start(wv_s[:], wv_r[:])
                nc.sync.dma_start(xsc1[:, 12:16, :], xT_r[:, 12:16, csl1])
                xs_tiles[1] = xsc1
            elif c == 1:
                pass  # tile + DMAs already emitted with chunk 0
            else:
                nc.sync.dma_start(xsc[:, 0:8, :], xT_r[:, 0:8, csl])
                nc.sync.dma_start(xsc[:, 8:16, :], xT_r[:, 8:16, csl])
            if c == 0:
                # all four Wo row-blocks are needed by the first s-tile's
                # output projection; stream them on the ACT hwdge queue
                for ci in range(G):
                    nc.scalar.dma_start(wo_s[:, ci, :], wo_r[:, ci, :])
            xs_tiles[c] = xsc

        def emit_proj(c):
            csl = slice(c * CH, (c + 1) * CH)
            xsc = xs_tiles.pop(c)
            # projections for this chunk: k first (attention needs it
            # soonest), then q heads, then v
            for r in [G] + list(range(G)) + [G + 1]:
                if r < G:
                    w_view = wq_s[:, :, r * HD : (r + 1) * HD]
                elif r == G:
                    w_view = wk_s[:, :, :]
                else:
                    w_view = wv_s[:, :, :]
                pp = ps.tile([128, CH], F32, tag="ps", name=f"pp_{c}_{r}")
                for d in range(DT):
                    nc.tensor.matmul(
                        pp[:],
                        lhsT=w_view[:, d, :],
                        rhs=xsc[:, d, :],
                        start=(d == 0),
                        stop=(d == DT - 1),
                    )
                if r < G:
                    rope_block(qt[0:64, r, csl], qt[64:128, r, csl], pp[:], c)
                elif r == G:
                    rope_block(kt[0:64, csl], kt[64:128, csl], pp[:], c)
                else:
                    nc.vector.tensor_copy(vt[:, csl], pp[:])

        def emit_vtrans(c):
            # V^T -> V for this chunk's 4 s-tiles (PE transpose)
            for st in range(4 * c, 4 * c + 4):
                tp = ps.tile([128, 128], BF16, tag="ps", name=f"tp_{st}")
                nc.tensor.transpose(
                    tp[:], vt[:, st * 128 : (st + 1) * 128], iden_s[:]
                )
                nc.vector.tensor_copy(v[:, st, :], tp[:])

        # ---- attention + output projection over 128-wide sq tiles ----
        # For each (st, sk) causal pair: ONE score matmul (4 heads packed on
        # the moving side, N=512), one exp, one AV matmul (all heads into one
        # PSUM bank), one denominator matmul. av lags scores by LAG pairs so
        # the PE never waits on the exp of its own pair. After an s-tile's
        # last AV, its normalize + Wo output projection are emitted inline.
        state = {"avp": None, "denp": None}
        ehs = {}

        def emit_sc(st, sk):
            stsl = slice(st * 128, (st + 1) * 128)
            scp = ps.tile([128, CH], F32, tag="ps", name=f"sc_{st}_{sk}")
            nc.tensor.matmul(
                scp[:],
                lhsT=kt[:, sk * 128 : (sk + 1) * 128],
                rhs=qt[:, :, stsl],
                start=True,
                stop=True,
            )
            eh = ehp.tile([128, G, 128], BF16, tag="eh", name=f"eh_{st}_{sk}")
            nc.scalar.activation(
                eh[:].rearrange("p h s -> p (h s)"),
                scp[:],
                mybir.ActivationFunctionType.Exp,
                scale=SCALE,
            )
            if sk == st:
                nc.vector.tensor_mul(
                    eh[:],
                    eh[:],
                    tri_s[:, None, :].to_broadcast((128, G, 128)),
                )
            ehs[(st, sk)] = eh

        def emit_av(st, sk):
            if sk == 0:
                state["avp"] = ps.tile([128, CH], F32, tag="ps", name=f"av_{st}")
                state["denp"] = ps.tile([128, CH], F32, tag="ps", name=f"den_{st}")
            avp, denp = state["avp"], state["denp"]
            eh = ehs.pop((st, sk))
            ehf = eh[:].rearrange("p h s -> p (h s)")
            nc.tensor.matmul(
                avp[:],
                lhsT=v[:, sk, :],
                rhs=ehf,
                start=(sk == 0),
                stop=(sk == st),
            )
            nc.tensor.matmul(
                denp[0:1, :],
                lhsT=ones_s[:, 0:1],
                rhs=ehf,
                start=(sk == 0),
                stop=(sk == st),
            )
            if sk == st:
                emit_norm(st, avp, denp)
                wo_queue.append(st)
                if len(wo_queue) > 2:
                    emit_wo(wo_queue.pop(0))

        wo_queue = []

        def emit_norm(st, avp, denp):
            stsl = slice(st * 128, (st + 1) * 128)
            rec = nrm.tile([1, CH], F32, tag="rec")
            nc.vector.reciprocal(rec[:], denp[0:1, :])
            bc = nrm.tile([128, CH], F32, tag="bc")
            nc.gpsimd.partition_broadcast(bc[:], rec[:])
            nc.vector.tensor_mul(
                att[:, :, stsl],
                avp[:].rearrange("p (h s) -> p h s", h=G),
                bc[:].rearrange("p (h s) -> p h s", h=G),
            )

        def emit_wo(st):
            stsl = slice(st * 128, (st + 1) * 128)
            ot = otp.tile([128, S], BF16, tag="ot", name=f"ot_{st}")
            for n in range(NCH):
                po = ps.tile([128, CH], F32, tag="ps", name=f"po_{st}_{n}")
                for ci in range(G):
                    nc.tensor.matmul(
                        po[:],
                        lhsT=att[:, ci, stsl],
                        rhs=wo_s[:, ci, n * CH : (n + 1) * CH],
                        start=(ci == 0),
                        stop=(ci == G - 1),
                    )
                nc.scalar.copy(ot[:, n * CH : (n + 1) * CH], po[:])
            nc.sync.dma_start(out[st * 128 : (st + 1) * 128, :], ot[:])

        emit_xs(0)
        emit_proj(0)
        emit_vtrans(0)
        emit_xs(1)
        emit_proj(1)
        emit_xs(2)
        for c in range(NCH):
            pairs = [
                (st, sk) for st in range(4 * c, 4 * c + 4) for sk in range(st + 1)
            ]
            for i, p in enumerate(pairs):
                emit_sc(*p)
                if i >= LAG:
                    emit_av(*pairs[i - LAG])
            if c + 3 < NCH:
                emit_xs(c + 3)
            if c + 2 < NCH:
                emit_proj(c + 2)
            for p in pairs[len(pairs) - LAG :]:
                emit_av(*p)
            if c + 1 < NCH:
                emit_vtrans(c + 1)
            # drain deferred output projections now: an in-order PE can't
            # reach them past the next chunk's first (rope-blocked) scores,
            # and they are exactly the work that hides the rope chain
            while wo_queue:
                emit_wo(wo_queue.pop(0))

    nc.compile()
    return nc


_EXEC_CACHE = None


def _get_exec():
    """Build the Bass program once and wrap it in a cached jitted shard_map.

    Inputs are concatenated on axis 0 across a (batch=2, group=4) device mesh
    so that batch-replicated tensors (weights, tables) and group-replicated
    tensors (x^T) are only transferred once each, and the jit/NEFF lowering
    happens a single time per process.
    """
    global _EXEC_CACHE
    if _EXEC_CACHE is not None:
        return _EXEC_CACHE

    import jax
    from jax.experimental.shard_map import shard_map
    from jax.sharding import Mesh, PartitionSpec

    from concourse import bass2jax, mybir as _mybir

    nc = build_kernel()
    bass2jax.install_neuronx_cc_hook()

    partition_name = (
        nc.partition_id_tensor.name if nc.partition_id_tensor is not None else None
    )
    in_names = []
    out_names = []
    out_avals = []
    for alloc in nc.m.functions[0].allocations:
        if not isinstance(alloc, _mybir.MemoryLocationSet):
            continue
        name = alloc.memorylocations[0].name
        if alloc.kind == "ExternalInput":
            if name != partition_name:
                in_names.append(name)
        elif alloc.kind == "ExternalOutput":
            out_names.append(name)
            out_avals.append(
                jax.core.ShapedArray(
                    tuple(alloc.tensor_shape), _mybir.dt.np(alloc.dtype)
                )
            )
    n_params = len(in_names)
    all_in_names = tuple(in_names) + tuple(out_names)
    if partition_name is not None:
        all_in_names = all_in_names + (partition_name,)

    def _body(*args):
        operands = list(args)
        if partition_name is not None:
            operands.append(bass2jax.partition_id_tensor())
        outs = bass2jax._bass_exec_p.bind(
            *operands,
            out_avals=tuple(out_avals),
            in_names=all_in_names,
            out_names=tuple(out_names),
            lowering_input_output_aliases=(),
            sim_require_finite=True,
            sim_require_nnan=True,
            nc=nc,
        )
        return tuple(outs)

    devices = jax.devices()[:N_CORES]
    mesh = Mesh(np.asarray(devices).reshape(B, NKV), ("b", "g"))
    # sharding of the axis-0-concatenated global inputs, in in_names order
    spec_by_name = {
        "xT": PartitionSpec("b"),  # [B*D, S]
        "wq": PartitionSpec("g"),  # [NKV*D, G*HD]
        "wk": PartitionSpec("g"),
        "wv": PartitionSpec("g"),
        "wo": PartitionSpec("g"),  # [NKV*G*HD, D]
        "csT": PartitionSpec(),  # replicated
        "tri": PartitionSpec(),
        "out": PartitionSpec(("b", "g")),
    }
    in_specs = tuple(spec_by_name[n] for n in tuple(in_names) + tuple(out_names))
    out_specs = tuple(spec_by_name[n] for n in out_names)
    donate = tuple(range(n_params, n_params + len(out_names)))
    sharded = jax.jit(
        shard_map(
            _body, mesh=mesh, in_specs=in_specs, out_specs=out_specs, check_rep=False
        ),
        donate_argnums=donate,
        keep_unused=True,
    )

    out_sharding = jax.sharding.NamedSharding(mesh, PartitionSpec(("b", "g")))
    zeros_fn = jax.jit(
        lambda: jax.numpy.zeros((N_CORES * S, D), jax.numpy.bfloat16),
        out_shardings=out_sharding,
    )

    _EXEC_CACHE = (sharded, tuple(in_names), mesh, zeros_fn)
    return _EXEC_CACHE


def _rope_tables():
    inv_freq = 1.0 / ROPE_BASE ** (np.arange(0, HD, 2, dtype=np.float32) / HD)
    t = np.arange(S, dtype=np.float32)
    freqs = np.outer(t, inv_freq)  # [S, HD/2]
    return (
        np.ascontiguousarray(np.cos(freqs).T.astype(np.float32)),
        np.ascontiguousarray(np.sin(freqs).T.astype(np.float32)),
    )


def make_global_inputs(x, Wq, Wk, Wv, Wo):
    """Axis-0-concatenated global arrays, keyed by DRAM tensor name."""
    bf16 = ml_dtypes.bfloat16
    # per-head evens-then-odds column permutation
    perm_h = np.concatenate([np.arange(0, HD, 2), np.arange(1, HD, 2)])
    perm_q = np.concatenate([h * HD + perm_h for h in range(NH)])
    perm_k = np.concatenate([h * HD + perm_h for h in range(NKV)])
    x = np.asarray(x)
    Wq_p = np.asarray(Wq)[:, perm_q].astype(bf16)
    Wk_p = np.asarray(Wk)[:, perm_k].astype(bf16)
    Wv = np.asarray(Wv).astype(bf16)
    cosT, sinT = _rope_tables()
    csT = np.concatenate([cosT, sinT], axis=0)  # [128, S]
    tri = (np.arange(128)[:, None] <= np.arange(128)[None, :]).astype(bf16)

    xT_g = np.concatenate([x[b].T for b in range(B)], axis=0).astype(bf16)
    wq_g = np.concatenate(
        [Wq_p[:, g * G * HD : (g + 1) * G * HD] for g in range(NKV)], axis=0
    )
    wk_g = np.concatenate([Wk_p[:, g * HD : (g + 1) * HD] for g in range(NKV)], axis=0)
    wv_g = np.concatenate([Wv[:, g * HD : (g + 1) * HD] for g in range(NKV)], axis=0)
    wo_g = np.asarray(Wo).astype(bf16)  # row-slice concat over g == Wo itself
    return {
        "xT": xT_g,
        "wq": wq_g,
        "wk": wk_g,
        "wv": wv_g,
        "wo": wo_g,
        "csT": csT,
        "tri": tri,
    }


def run_global(inputs_g, time_exec=False):
    """Run the kernel on pre-built global input arrays; returns [B, S, D] f32."""
    import jax
    import time as _time

    sharded, in_names, mesh, zeros_fn = _get_exec()
    args = [inputs_g[n] for n in in_names]
    out_g = sharded(*args, zeros_fn())
    if time_exec:
        # device_put inputs once, then time execution only
        from jax.sharding import NamedSharding

        dev_args = [
            jax.device_put(a, NamedSharding(mesh, s))
            for a, s in zip(args, sharded_in_specs())
        ]
        jax.block_until_ready(dev_args)
        times = []
        for _ in range(5):
            z = zeros_fn()
            jax.block_until_ready(z)
            t0 = _time.perf_counter()
            o = sharded(*dev_args, z)
            jax.block_until_ready(o)
            times.append(_time.perf_counter() - t0)
        print(f"exec-only times (ms): {[f'{t*1e3:.2f}' for t in times]}")
        out_g = o
    out = (
        np.asarray(out_g)
        .astype(np.float32)
        .reshape(B, NKV, S, D)
        .sum(axis=1, dtype=np.float32)
    )
    return out


def sharded_in_specs():
    from jax.sharding import PartitionSpec

    spec_by_name = {
        "xT": PartitionSpec("b"),
        "wq": PartitionSpec("g"),
        "wk": PartitionSpec("g"),
        "wv": PartitionSpec("g"),
        "wo": PartitionSpec("g"),
        "csT": PartitionSpec(),
        "tri": PartitionSpec(),
    }
    _, in_names, _, _ = _get_exec()
    return [spec_by_name[n] for n in in_names]


def kernel(x, mask, Wq, Wk, Wv, Wo):
    inputs_g = make_global_inputs(x, Wq, Wk, Wv, Wo)
    return run_global(inputs_g)
